# revision 30
# baseline (speedup 1.0000x reference)
"""GRU cell kernel for Trainium2, data-parallel over 8 NeuronCores.

Reference computation (B=4096, I=H=2048, C=I+H=4096):
    combined   = [x, h]                                   [B, C]
    to_update  = sigmoid(combined @ W_update.T + b_u)     [B, H]
    to_select  = sigmoid(combined @ W_select.T + b_s)     [B, H]
    updated    = h * to_update
    new_comb   = [x, updated]
    predictions= tanh(new_comb @ W_predict.T + b_p)       [B, H]
    h_new      = h * (1 - to_select) + predictions * to_select

Sharding: batch split 8 ways (512 rows/core), weights replicated.
On-chip layout is [feature, batch] (transposed): weight tiles
[128c, 128h] are the stationary matmul operand, activation tiles
[128c, 512b] the moving operand. Matmuls in bf16 with fp32 PSUM;
gates and the final blend in fp32.

fp8 DoubleRow for k-tiles 8..11: those four contraction tiles (x
region, shared by all three gemms) run as two fp8-e4m3 DoubleRow
pairs — half the matmul instructions at double contraction per pass.
Scale is split (weights x8, activations /8) so the fp8 partial
products accumulate at natural scale into the same PSUM bank as the
bf16 tiles. Measured end-to-end error 0.0143 vs the 0.02 gate
(deterministic inputs).

Schedule (per core):
- The first 8 select-gate groups accumulate chunk-by-chunk across all
  8 PSUM banks, in lockstep with the arrival of the combined-input
  chunks (weight columns for those groups stream in matching chunks),
  so the PE starts ~2us in and never waits on the initial input load.
- select runs first: 1-s comes free from a second sigmoid with
  scale=-1, and hm = h*(1-s) is precomputed on the idle vector engine,
  shrinking the final per-tile tail to tanh -> mul -> add -> DMA.
- The last predict tile runs as two sequential 256-wide groups so the
  first half's epilogue overlaps the second half's matmuls and the
  closing drain is half-sized.
"""

from contextlib import ExitStack

import numpy as np
import ml_dtypes

import concourse.tile as tile
import concourse.mybir as mybir
from concourse import bacc
from concourse.bass_utils import run_bass_kernel_spmd

BF16 = mybir.dt.bfloat16
F8 = mybir.dt.float8e4
F32 = mybir.dt.float32
NPBF16 = ml_dtypes.bfloat16
NPF8 = ml_dtypes.float8_e4m3fn
DR = mybir.MatmulPerfMode.DoubleRow

B, I, H = 4096, 2048, 2048
C = I + H
NCORES = 8
BS = B // NCORES            # 512 batch rows per core
P = 128                     # SBUF partitions
HT = H // P                 # 16 output-row tiles
IT = I // P                 # 16 x feature tiles
CT = C // P                 # 32 contraction tiles
CHT = 4                     # k-tiles per input DMA chunk
NCH = CT // CHT             # 8 combined-input chunks
NW = 7                      # groups in the chunk-paced startup wave
KQ0 = 8                     # first fp8 k-tile (k-tiles KQ0..KQ0+3 are fp8)
NPAIR = 2                   # fp8 DoubleRow pairs (2 k-tiles each)
QCH = KQ0 // CHT            # the input chunk covered by fp8 (chunk 2)
NKB = CT - 2 * NPAIR        # bf16 k-tiles per weight block (28)
WSCALE = 8.0                # fp8 weights x8, activations /8
ACT_F = mybir.ActivationFunctionType

_PROGRAM = None


def _build_program():
    nc = bacc.Bacc("TRN2")

    # chunk-major bf16 inputs (chunk QCH unused; its k-tiles ride in xq)
    xh = nc.dram_tensor("xh", [NCH, P, CHT * BS], BF16, kind="ExternalInput")
    # fp8 activation pairs: [pair, k, j, batch] = x.T[(KQ0+2p+j)*128+k]/8
    xq = nc.dram_tensor("xq", [NPAIR, P, 2, BS], F8, kind="ExternalInput")
    # bf16 weight blocks without the fp8 k-tiles: k order [0..7, 12..31]
    Wu = nc.dram_tensor("Wu", [HT, P, NKB * P], BF16, kind="ExternalInput")
    Ws = nc.dram_tensor("Ws", [HT, P, NKB * P], BF16, kind="ExternalInput")
    Wp = nc.dram_tensor("Wp", [HT, P, NKB * P], BF16, kind="ExternalInput")
    # fp8 weight pairs: [i, k, j, p*128+m] = W[i*128+m, (KQ0+2p+j)*128+k]*8
    Wqu = nc.dram_tensor("Wqu", [HT, P, 2, 2 * P], F8, kind="ExternalInput")
    Wqs = nc.dram_tensor("Wqs", [HT, P, 2, 2 * P], F8, kind="ExternalInput")
    Wqp = nc.dram_tensor("Wqp", [HT, P, 2, 2 * P], F8, kind="ExternalInput")
    # wave weights for groups 1..6 fused in pairs per chunk:
    # [jj, g, k, 0:512]=Ws_blk[1+2g, cw(j):+512], [512:1024]=Ws_blk[2+2g]
    Wsw = nc.dram_tensor("Wsw", [NCH - 1, (NW - 1) // 2, P, 2 * CHT * P],
                         BF16, kind="ExternalInput")
    # fp8 wave weights for groups 0..6 fused: [k, j, i*256+p*128+m]
    Wq8w = nc.dram_tensor("Wq8w", [P, 2, NW * NPAIR * P], F8,
                          kind="ExternalInput")
    # bias columns: [bu | bs | bp | -bs], HT each
    bias = nc.dram_tensor("bias", [P, 4 * HT], F32, kind="ExternalInput")
    out = nc.dram_tensor("out", [HT, P, BS], F32, kind="ExternalOutput")

    def wcol(k):
        """column offset of bf16 k-tile k in the packed 28-tile block"""
        return (k if k < KQ0 else k - 2 * NPAIR) * P

    with tile.TileContext(nc) as tc, ExitStack() as ctx:
        singles = ctx.enter_context(tc.tile_pool(name="singles", bufs=1))
        wavep = ctx.enter_context(tc.tile_pool(name="wavep", bufs=10))
        wpool = ctx.enter_context(tc.tile_pool(name="wpool", bufs=4))
        fqpool = ctx.enter_context(tc.tile_pool(name="fqpool", bufs=6))
        pspool = ctx.enter_context(tc.tile_pool(name="ps", bufs=8, space="PSUM"))
        work = ctx.enter_context(tc.tile_pool(name="work", bufs=2))

        # PE warm-up: zero-input matmuls fill the dead time while the first
        # DMAs land, so the HAM clock gate is released before real work
        warm = singles.tile([P, P], BF16, name="warm")
        nc.vector.memset(warm[:], 0)
        wps = pspool.tile([P, P], F32, tag="ps", name="wps")
        for _ in range(29):
            nc.tensor.matmul(wps, warm[:], warm[:], start=True, stop=True)

        # --- input + wave-weight DMAs, interleaved per chunk ---
        # chunk 0 splits into two half-tiles, and its first weight
        # sub-block leads the ring, so the first matmul starts ~2.5us in
        comb_ch = []
        wsub = [[None] * NCH for _ in range(NW)]
        wq8w = [None] * NW
        aq = [None] * NPAIR
        c0 = [
            singles.tile([P, 2 * BS], BF16, name=f"comb0{a}", tag=f"comb0{a}")
            for a in range(2)
        ]
        bf16_chunks = [j for j in range(NCH) if j != QCH]
        for jj, j in enumerate(bf16_chunks):
            if j == 0:
                w = wavep.tile([P, CHT * P], BF16, tag="w0", name="w0")
                nc.sync.dma_start(w[:], Ws[0, :, 0:CHT * P])
                wsub[0][0] = w
                nc.sync.dma_start(c0[0][:], xh[0, :, 0:2 * BS])
                nc.sync.dma_start(c0[1][:], xh[0, :, 2 * BS:CHT * BS])
                comb_ch.append(None)
            else:
                t = singles.tile(
                    [P, CHT * BS], BF16, name=f"comb{j}", tag=f"comb{j}"
                )
                nc.sync.dma_start(t[:], xh[j])
                while len(comb_ch) < j:
                    comb_ch.append(None)
                comb_ch.append(t)
                w = wavep.tile([P, CHT * P], BF16, tag="w0", name="w0")
                cw = wcol(j * CHT)
                nc.sync.dma_start(w[:], Ws[0, :, cw:cw + CHT * P])
                wsub[0][j] = w
            for g in range((NW - 1) // 2):
                w = wavep.tile([P, 2 * CHT * P], BF16, tag="wsub", name="wsub")
                nc.sync.dma_start(w[:], Wsw[jj, g])
                wsub[1 + 2 * g][j] = w[:, 0:CHT * P]
                wsub[2 + 2 * g][j] = w[:, CHT * P:2 * CHT * P]
        # fp8 chunk last on the ring: its wave matmuls run at the wave's
        # end (accumulation is order-free) and are 4x cheaper, so the bf16
        # chunks' data lands earlier and the wave never stalls mid-stream
        for p2 in range(NPAIR):
            t = singles.tile([P, 2, BS], F8, name=f"aq{p2}")
            nc.sync.dma_start(t[:], xq[p2])
            aq[p2] = t
        wq8all = singles.tile([P, 2, NW * NPAIR * P], F8, name="wq8all")
        nc.sync.dma_start(wq8all[:], Wq8w[:])
        for i in range(NW):
            wq8w[i] = wq8all[:, :, i * NPAIR * P:(i + 1) * NPAIR * P]

        bias_sb = singles.tile([P, 4 * HT], F32, name="bias_sb")
        nc.sync.dma_start(bias_sb[:], bias[:])

        def comb_t(k):
            if k < CHT:
                return c0[k // 2][:, (k % 2) * BS:(k % 2 + 1) * BS]
            return comb_ch[k // CHT][:, (k % CHT) * BS:(k % CHT + 1) * BS]

        sel = [
            singles.tile([P, BS], F32, name=f"sel{i}", tag=f"sel{i}")
            for i in range(HT)
        ]
        hm = [
            singles.tile([P, BS], F32, name=f"hm{i}", tag=f"hm{i}")
            for i in range(HT)
        ]
        upd = [
            singles.tile([P, BS], BF16, name=f"upd{i}", tag=f"upd{i}")
            for i in range(HT)
        ]

        # --- startup wave: select groups 0..7, chunk-paced ---
        ps_w = [
            pspool.tile([P, BS], F32, tag="ps", name=f"psw{i}") for i in range(NW)
        ]
        for j in bf16_chunks:
            for i in range(NW):
                for m in range(CHT):
                    k = j * CHT + m
                    nc.tensor.matmul(
                        ps_w[i],
                        wsub[i][j][:, m * P:(m + 1) * P],
                        comb_t(k),
                        start=(k == 0),
                        stop=False,
                    )
        for i in range(NW):
            for p2 in range(NPAIR):
                nc.tensor.matmul(
                    ps_w[i],
                    wq8w[i][:, :, p2 * P:(p2 + 1) * P],
                    aq[p2][:],
                    start=False,
                    stop=(p2 == NPAIR - 1),
                    perf_mode=DR,
                )

        def sel_epilogue(i, ps):
            nc.scalar.activation(
                sel[i][:], ps[:], ACT_F.Sigmoid, bias=bias_sb[:, HT + i:HT + i + 1]
            )
            sc = work.tile([P, BS], F32, tag="sc", name="sc")
            nc.scalar.activation(
                sc[:], ps[:], ACT_F.Sigmoid,
                bias=bias_sb[:, 3 * HT + i:3 * HT + i + 1], scale=-1.0,
            )
            nc.vector.tensor_mul(hm[i][:], comb_t(IT + i), sc[:])

        for i in range(NW):
            sel_epilogue(i, ps_w[i])

        def gemm(W, Wq, rhs_of_k, i, cols=None):
            """psum = sum_k W_tile[i].T @ rhs; bf16 + 2 fp8 DoubleRow pairs"""
            wblk = wpool.tile([P, NKB * P], BF16, tag="wblk", name="wblk")
            nc.sync.dma_start(wblk[:], W[i])
            wq8 = fqpool.tile([P, 2, 2 * P], F8, tag="wq8", name="wq8")
            nc.sync.dma_start(wq8[:], Wq[i])
            n_ = BS if cols is None else cols.stop - cols.start
            ps = pspool.tile([P, n_], F32, tag="ps", name="ps")

            def rhs(k):
                r = rhs_of_k(k)
                return r if cols is None else r[:, cols]

            for k in range(KQ0):
                nc.tensor.matmul(
                    ps, wblk[:, wcol(k):wcol(k) + P], rhs(k),
                    start=(k == 0), stop=False,
                )
            for p2 in range(NPAIR):
                a = aq[p2][:] if cols is None else aq[p2][:, :, cols]
                nc.tensor.matmul(
                    ps, wq8[:, :, p2 * P:(p2 + 1) * P], a,
                    start=False, stop=False, perf_mode=DR,
                )
            for k in range(KQ0 + 2 * NPAIR, CT):
                nc.tensor.matmul(
                    ps, wblk[:, wcol(k):wcol(k) + P], rhs(k),
                    start=False, stop=(k == CT - 1),
                )
            return ps

        # --- remaining select gates ---
        for i in range(NW, HT):
            ps = gemm(Ws, Wqs, comb_t, i)
            sel_epilogue(i, ps)

        # --- update gates: upd = h * sigmoid(z_u) (bf16, feeds matmul 3) ---
        for i in range(HT):
            ps = gemm(Wu, Wqu, comb_t, i)
            u = work.tile([P, BS], BF16, tag="u", name="u")
            nc.scalar.activation(
                u[:], ps[:], ACT_F.Sigmoid, bias=bias_sb[:, i:i + 1]
            )
            nc.vector.tensor_mul(upd[i][:], comb_t(IT + i), u[:])

        def newcomb_t(k):
            return comb_t(k) if k < IT else upd[k - IT][:]

        # --- predictions + blend: h_new = hm + sel * tanh(z_p) ---
        for i in range(HT - 1):
            ps = gemm(Wp, Wqp, newcomb_t, i)
            p_t = work.tile([P, BS], F32, tag="p", name="p_t")
            nc.scalar.activation(
                p_t[:], ps[:], ACT_F.Tanh, bias=bias_sb[:, 2 * HT + i:2 * HT + i + 1]
            )
            nc.vector.tensor_mul(p_t[:], p_t[:], sel[i][:])
            o = work.tile([P, BS], F32, tag="o", name="o")
            nc.vector.tensor_add(o[:], hm[i][:], p_t[:])
            nc.sync.dma_start(out[i], o[:])

        # last tile as two sequential 256-wide groups: half-a epilogue
        # overlaps half-b matmuls, and the closing drain is half-sized
        i = HT - 1
        for c0_ in (0, 256):
            cols = slice(c0_, c0_ + 256)
            ps = gemm(Wp, Wqp, newcomb_t, i, cols=cols)
            p_t = work.tile([P, 256], F32, tag="p", name="p_h")
            nc.scalar.activation(
                p_t[:], ps[:], ACT_F.Tanh, bias=bias_sb[:, 2 * HT + i:2 * HT + i + 1]
            )
            nc.vector.tensor_mul(p_t[:], p_t[:], sel[i][:, cols])
            o = work.tile([P, 256], F32, tag="o", name="o_h")
            nc.vector.tensor_add(o[:], hm[i][:, cols], p_t[:])
            nc.sync.dma_start(out[i, :, cols], o[:])

    nc.finalize()
    return nc


def _get_program():
    global _PROGRAM
    if _PROGRAM is None:
        _PROGRAM = _build_program()
    return _PROGRAM


_KEEP = np.r_[0:KQ0, KQ0 + 2 * NPAIR:CT]


def _wcol_np(j):
    k = j * CHT
    c = (k if k < KQ0 else k - 2 * NPAIR) * P
    return slice(c, c + CHT * P)


def _pack_weight(w):
    """[H, C] fp32 -> [HT, P, 28*P] bf16, k-tiles [0..7, 12..31].

    [i, k, n*128+m] = W[i*128+m, kt(n)*128+k]: column window n*128 is the
    stationary operand (lhsT = W.T tile) for the n-th retained k-tile.
    """
    wb = np.asarray(w, dtype=np.float32).astype(NPBF16)
    return np.ascontiguousarray(
        wb.reshape(HT, P, CT, P)[:, :, _KEEP, :]
        .transpose(0, 3, 2, 1).reshape(HT, P, NKB * P)
    )


def _pack_weight_fp8(w):
    """[H, C] fp32 -> [HT, P, 2, 2*P] fp8 pairs for k-tiles KQ0..KQ0+3.

    [i, k, j, p*128+m] = W[i*128+m, (KQ0+2p+j)*128+k] * WSCALE
    """
    wf = np.asarray(w, dtype=np.float32).reshape(HT, P, CT, P)
    sub = wf[:, :, KQ0:KQ0 + 2 * NPAIR, :] * WSCALE     # [i, m, t, k]
    sub = sub.reshape(HT, P, NPAIR, 2, P)               # [i, m, p, j, k]
    return np.ascontiguousarray(
        sub.transpose(0, 4, 3, 2, 1).reshape(HT, P, 2, NPAIR * P).astype(NPF8)
    )


def _chunk(tiles):
    """[T, P, BS] -> [T//CHT, P, CHT*BS] chunk-major (contiguous per chunk)."""
    T = tiles.shape[0]
    return np.ascontiguousarray(
        tiles.reshape(T // CHT, CHT, P, BS).transpose(0, 2, 1, 3)
        .reshape(T // CHT, P, CHT * BS)
    )


def _prep_inputs(x, h, W_update, b_update, W_select, b_select, W_predict, b_predict):
    x = np.asarray(x, dtype=np.float32)
    h = np.asarray(h, dtype=np.float32)

    Wu = _pack_weight(W_update)
    Ws = _pack_weight(W_select)
    Wp = _pack_weight(W_predict)
    Wqu = _pack_weight_fp8(W_update)
    Wqs = _pack_weight_fp8(W_select)
    Wqp = _pack_weight_fp8(W_predict)
    bf16_chunks = [j for j in range(NCH) if j != QCH]
    Wsw = np.stack([
        np.stack([
            np.concatenate([Ws[1 + 2 * g, :, _wcol_np(j)],
                            Ws[2 + 2 * g, :, _wcol_np(j)]], axis=1)
            for g in range((NW - 1) // 2)
        ])
        for j in bf16_chunks
    ])
    Wsw = np.ascontiguousarray(Wsw)
    Wq8w = np.ascontiguousarray(
        np.concatenate([Wqs[i] for i in range(NW)], axis=-1)
    )
    bu = np.asarray(b_update, dtype=np.float32).reshape(HT, P).T
    bs = np.asarray(b_select, dtype=np.float32).reshape(HT, P).T
    bp = np.asarray(b_predict, dtype=np.float32).reshape(HT, P).T
    bias = np.ascontiguousarray(np.concatenate([bu, bs, bp, -bs], axis=1))

    in_maps = []
    for c in range(NCORES):
        rows = slice(c * BS, (c + 1) * BS)
        xT = x[rows].T
        xh = np.concatenate(
            [_chunk(xT.astype(NPBF16).reshape(IT, P, BS)),
             _chunk(h[rows].T.astype(NPBF16).reshape(HT, P, BS))], axis=0
        )
        # fp8 pairs: [p, k, j, b] = xT[(KQ0+2p+j)*128+k, b] / WSCALE
        xqt = (xT[KQ0 * P:(KQ0 + 2 * NPAIR) * P] / WSCALE).reshape(
            NPAIR, 2, P, BS
        )
        xq = np.ascontiguousarray(xqt.transpose(0, 2, 1, 3).astype(NPF8))
        in_maps.append(
            {
                "xh": xh,
                "xq": xq,
                "Wu": Wu,
                "Ws": Ws,
                "Wp": Wp,
                "Wqu": Wqu,
                "Wqs": Wqs,
                "Wqp": Wqp,
                "Wsw": Wsw,
                "Wq8w": Wq8w,
                "bias": bias,
            }
        )
    return in_maps


def kernel(x, h, W_update, b_update, W_select, b_select, W_predict, b_predict,
           _trace=False):
    nc = _get_program()
    in_maps = _prep_inputs(
        x, h, W_update, b_update, W_select, b_select, W_predict, b_predict
    )
    res = run_bass_kernel_spmd(
        nc, in_maps, core_ids=list(range(NCORES)), trace=_trace
    )
    h_new = np.empty((B, H), dtype=np.float32)
    for c in range(NCORES):
        rows = slice(c * BS, (c + 1) * BS)
        h_new[rows] = res.results[c]["out"].reshape(H, BS).T
    if _trace:
        return h_new, res
    return h_new
